# revision 23
# baseline (speedup 1.0000x reference)
"""Trainium2 Bass kernel for AverageSpanExtractor (segment mean over spans).

Math note: the reference's masked softmax over all-ones logits reduces
exactly to a mean over the span tokens [start, end):
    out[b, n, :] = mean(sequence_tensor[b, start:end, :]).

Strategy (8 cores, batch-parallel — one batch element per core), built
around sorted-span segment matmuls instead of prefix sums + gathers
(measured: ANY indexed fetch of 3k rows costs >=20us on this part —
SWDGE desc-gen ~9ns/idx, ap_gather/indirect_copy ~30ns/idx):

  1. HOST: sort each batch's spans by start. A 128-span chunk of the
     sorted order covers a ~640-token window, i.e. 5-7 of the 32
     128-token blocks. Window bounds (B0_j, K_j) are unioned across
     the 8 cores so one SPMD program serves all; the nc is built (and
     cached) per span-structure, so bounds are exact for the given
     inputs, correct for any.
  2. DEVICE: per (chunk j, window block b), build the binary indicator
     M[i, t] = (s_i <= t < e_i) with two fused DVE/gpsimd compare ops
     against an iota row (host supplies per-window shifted bounds),
     PE-transpose it to token-major, and accumulate
        out_j += M_T.T @ x_block          (f16 inputs, f32 PSUM)
     Sequence blocks are DMA-streamed f32 and cast f16 on the scalar/
     vector engines; chunk matmuls chase the loads.
  3. Scale rows by 1/w (f32, per-partition) during the PSUM->SBUF
     copy, store contiguous (sorted order). HOST: unpermute rows.

Precision: binary f16 indicator is exact; x quantized to f16
(2^-11) => ~3e-4 global rel err. No prefix-difference cancellation.
"""

import numpy as np

B, S, D = 8, 4096, 256
N_SPANS = 1024
P = 128
NBLK = S // P
JG = N_SPANS // P      # 8 span chunks of 128

_cache = {"key": None, "nc": None, "windows": None}


def _plan_windows(si):
    """Per-chunk sorted-span block windows, unioned across cores.

    Returns (perms [B,1024], windows: list per j of (B0, K)), plus
    sorted s/e arrays [B, 1024].
    """
    perms = np.empty((B, N_SPANS), dtype=np.int64)
    ss = np.empty((B, N_SPANS), dtype=np.int64)
    ee = np.empty((B, N_SPANS), dtype=np.int64)
    for b in range(B):
        perm = np.argsort(si[b, :, 0], kind="stable")
        perms[b] = perm
        ss[b] = si[b, perm, 0]
        ee[b] = si[b, perm, 1]
    windows = []
    for j in range(JG):
        b0 = NBLK
        b1 = 0
        for b in range(B):
            cs = ss[b, j * P : (j + 1) * P]
            ce = ee[b, j * P : (j + 1) * P]
            b0 = min(b0, int(cs.min()) >> 7)
            b1 = max(b1, (int(ce.max()) - 1) >> 7)
        windows.append((b0, b1 - b0 + 1))
    return perms, windows, ss, ee


def build_nc(windows):
    import concourse.bacc as bacc
    import concourse.mybir as mybir
    from concourse.tile import TileContext
    from concourse.masks import make_identity

    f32 = mybir.dt.float32
    f16 = mybir.dt.float16
    Alu = mybir.AluOpType
    Act = mybir.ActivationFunctionType

    NW = sum(k for _, k in windows)

    nc = bacc.Bacc(None, target_bir_lowering=False, debug=False, num_devices=B)
    seq = nc.declare_dram_parameter("seq", [S, D], f32, isOutput=False)
    # Per-window shifted span bounds (f32): column w of window (j, b)
    # holds s_sorted[128j+p] - 128*(B0_j+b) (resp. e).
    swin = nc.declare_dram_parameter("swin", [P, NW], f32, isOutput=False)
    ewin = nc.declare_dram_parameter("ewin", [P, NW], f32, isOutput=False)
    # 1/w per sorted span, chunk-major: [p, j].
    wrec = nc.declare_dram_parameter("wrec", [P, JG], f32, isOutput=False)
    out = nc.declare_dram_parameter("out", [N_SPANS, D], f32, isOutput=True)

    with TileContext(nc) as tc:
        with (
            tc.tile_pool(name="const", bufs=1) as const_pool,
            tc.tile_pool(name="x", bufs=8) as x_pool,
            tc.tile_pool(name="m", bufs=6) as m_pool,
            tc.tile_pool(name="ps", bufs=4, space="PSUM") as ps_pool,
            tc.tile_pool(name="pst", bufs=4, space="PSUM") as pst_pool,
            tc.tile_pool(name="misc", bufs=1) as misc_pool,
            tc.tile_pool(name="res", bufs=3) as res_pool,
        ):
            identh = const_pool.tile([P, P], f16)
            make_identity(nc, identh[:])
            iota = const_pool.tile([P, P], mybir.dt.int32)
            nc.gpsimd.iota(iota[:], pattern=[[1, P]], base=0, channel_multiplier=0)
            iotaF = const_pool.tile([P, P], f32)
            nc.gpsimd.tensor_copy(out=iotaF[:], in_=iota[:])

            # bounds ride the ACT ring; seq loads own the SP ring
            SW = misc_pool.tile([P, NW], f32)
            nc.scalar.dma_start(out=SW[:], in_=swin[:])
            EW = misc_pool.tile([P, NW], f32)
            nc.scalar.dma_start(out=EW[:], in_=ewin[:])
            WR = misc_pool.tile([P, JG], f32)
            nc.scalar.dma_start(out=WR[:], in_=wrec[:])

            GB = 4
            NG = NBLK // GB
            XH = misc_pool.tile([P, NBLK * D], f16)
            MTbig = misc_pool.tile([P, NW, P], f16)

            def emit_load(g):
                t0 = g * GB * P
                bigx = x_pool.tile([P, GB * D], f32)
                nc.sync.dma_start(
                    out=bigx[:],
                    in_=seq[t0 : t0 + GB * P, :].rearrange(
                        "(m p) d -> p m d", p=P
                    ),
                )
                return bigx

            bigxs = [emit_load(g) for g in range(NG)]

            def emit_cast(g):
                nc.scalar.activation(
                    out=XH[:, g * GB * D : (g + 1) * GB * D],
                    in_=bigxs[g][:], func=Act.Copy,
                )

            wbase = [0] * JG
            w = 0
            for j in range(JG):
                wbase[j] = w
                w += windows[j][1]

            def emit_compares(j):
                b0, kj = windows[j]
                for bb in range(kj):
                    ww = wbase[j] + bb
                    A = m_pool.tile([P, P], f16, name=f"A{ww}")
                    nc.vector.tensor_scalar(
                        out=A[:], in0=iotaF[:], scalar1=SW[:, ww : ww + 1],
                        scalar2=None, op0=Alu.is_ge,
                    )
                    M = m_pool.tile([P, P], f16, name=f"M{ww}")
                    nc.vector.scalar_tensor_tensor(
                        out=M[:], in0=iotaF[:], scalar=EW[:, ww : ww + 1],
                        in1=A[:], op0=Alu.is_lt, op1=Alu.mult,
                    )
                    yield M

            def emit_transposes(j):
                for bb, M in enumerate(emit_compares(j)):
                    ww = wbase[j] + bb
                    pst = pst_pool.tile([P, P], f16)
                    nc.tensor.transpose(out=pst[:], in_=M[:], identity=identh[:])
                    if ww % 2 == 0:
                        nc.vector.tensor_copy(out=MTbig[:, ww, :], in_=pst[:])
                    else:
                        nc.scalar.activation(
                            out=MTbig[:, ww, :], in_=pst[:], func=Act.Copy
                        )

            def emit_mms(j):
                b0, kj = windows[j]
                ps = ps_pool.tile([P, D], f32)
                for bb in range(kj):
                    blk = b0 + bb
                    nc.tensor.matmul(
                        out=ps[:],
                        lhsT=MTbig[:, wbase[j] + bb, :],
                        rhs=XH[:, blk * D : (blk + 1) * D],
                        start=(bb == 0), stop=(bb == kj - 1),
                    )
                rj = res_pool.tile([P, D], f32)
                nc.scalar.activation(
                    out=rj[:], in_=ps[:], func=Act.Copy,
                    scale=WR[:, j : j + 1],
                )
                oj = out[:].rearrange("(c p) d -> p c d", p=P)[:, j, :]
                nc.scalar.dma_start(out=oj, in_=rj[:])

            # pipeline: casts stay 2 groups ahead; transposes 1 chunk ahead
            emit_cast(0)
            emit_cast(1)
            emit_transposes(0)
            for j in range(JG):
                if j + 2 < NG:
                    emit_cast(j + 2)
                if j + 1 < JG:
                    emit_transposes(j + 1)
                emit_mms(j)
    nc.finalize()
    return nc


def _make_in_maps(sequence_tensor, si, perms, windows, ss, ee):
    seq = np.ascontiguousarray(np.asarray(sequence_tensor), dtype=np.float32)
    NW = sum(k for _, k in windows)
    in_maps = []
    for b in range(B):
        sw = np.empty((P, NW), dtype=np.float32)
        ew = np.empty((P, NW), dtype=np.float32)
        w = 0
        for j in range(JG):
            b0, kj = windows[j]
            cs = ss[b, j * P : (j + 1) * P].astype(np.float32)
            ce = ee[b, j * P : (j + 1) * P].astype(np.float32)
            for bb in range(kj):
                base = 128.0 * (b0 + bb)
                sw[:, w] = cs - base
                ew[:, w] = ce - base
                w += 1
        wr = (
            1.0
            / (ee[b] - ss[b]).astype(np.float32)
        ).reshape(JG, P).T.copy()
        in_maps.append({"seq": seq[b], "swin": sw, "ewin": ew, "wrec": wr})
    return in_maps


def kernel(sequence_tensor, span_indices):
    from concourse.bass_utils import run_bass_kernel_spmd

    si = np.asarray(span_indices)
    assert si.shape == (B, N_SPANS, 2)
    key = si.tobytes()
    if _cache["key"] != key:
        perms, windows, ss, ee = _plan_windows(si)
        _cache.update(
            key=key, nc=build_nc(windows),
            plan=(perms, windows, ss, ee),
        )
    perms, windows, ss, ee = _cache["plan"]
    in_maps = _make_in_maps(sequence_tensor, si, perms, windows, ss, ee)
    res = run_bass_kernel_spmd(_cache["nc"], in_maps, list(range(B)))
    full = np.empty((B, N_SPANS, D), dtype=np.float32)
    for b in range(B):
        # device row i (sorted order) -> original span perms[b][i]
        full[b, perms[b], :] = res.results[b]["out"]
    return full


# revision 24
# speedup vs baseline: 1.3206x; 1.3206x over previous
"""Trainium2 Bass kernel for AverageSpanExtractor (segment mean over spans).

Math note: the reference's masked softmax over all-ones logits reduces
exactly to a mean over the span tokens [start, end):
    out[b, n, :] = mean(sequence_tensor[b, start:end, :]).

Strategy (8 cores, batch-parallel — one batch element per core), built
around sorted-span segment matmuls. Indexed-fetch approaches (SWDGE
gather, ap_gather, indirect_copy) all bottom out at >=20us for the 3k
random row fetches this problem needs; the PE is power-throttled to
~1.2 GHz with ~180ns fixed cost per matmul, so the design minimizes PE
instruction count:

  1. HOST: sort each batch's spans by start. A 128-span chunk of the
     sorted order covers a ~640-token window (5-7 of the 32 128-token
     blocks). Window bounds are unioned across the 8 cores so one SPMD
     program serves all; the nc is built (and cached) per
     span-structure — exact for the given inputs, correct for any.
  2. HOST also ships, per window, the span bounds shifted into the
     window and clipped to int8 ([s-128*blk, e-1-128*blk] clipped to
     [-128,127], or a (127,-128) sentinel when the span misses the
     window), replicated across partitions, s/e interleaved: 1.5 MB.
  3. DEVICE: per window, the token-major indicator
        MT[t, i] = (s8[i] <= t) * (e8m1[i] >= t)
     builds with two fused DVE ops (compare against the partition-index
     column) — no PE transposes. Per chunk, K_j matmuls accumulate
        out_j += MT.T @ x_block        (f16, f32 PSUM)
     chasing the f32->f16 casts of the streamed sequence.
  4. Scale rows by 1/w during the PSUM->SBUF copy (scalar engine,
     activation scale), store contiguous; HOST unpermutes rows.

Precision: binary f16 indicator is exact; x quantized to f16 (2^-11)
=> ~2e-4 global rel err.
"""

import numpy as np

B, S, D = 8, 4096, 256
N_SPANS = 1024
P = 128
NBLK = S // P
JG = N_SPANS // P      # 8 span chunks of 128

_cache = {"key": None}


def _plan_windows(si):
    """Sorted-span chunk block windows, unioned across cores."""
    perms = np.empty((B, N_SPANS), dtype=np.int64)
    ss = np.empty((B, N_SPANS), dtype=np.int64)
    ee = np.empty((B, N_SPANS), dtype=np.int64)
    for b in range(B):
        perm = np.argsort(si[b, :, 0], kind="stable")
        perms[b] = perm
        ss[b] = si[b, perm, 0]
        ee[b] = si[b, perm, 1]
    windows = []
    for j in range(JG):
        b0 = NBLK
        b1 = 0
        for b in range(B):
            cs = ss[b, j * P : (j + 1) * P]
            ce = ee[b, j * P : (j + 1) * P]
            b0 = min(b0, int(cs.min()) >> 7)
            b1 = max(b1, (int(ce.max()) - 1) >> 7)
        windows.append((b0, b1 - b0 + 1))
    return perms, windows, ss, ee


def build_nc(windows):
    import concourse.bacc as bacc
    import concourse.mybir as mybir
    from concourse.tile import TileContext

    f32 = mybir.dt.float32
    f16 = mybir.dt.float16
    i8 = mybir.dt.int8
    i32 = mybir.dt.int32
    Alu = mybir.AluOpType
    Act = mybir.ActivationFunctionType

    NW = sum(k for _, k in windows)

    nc = bacc.Bacc(None, target_bir_lowering=False, debug=False, num_devices=B)
    seq = nc.declare_dram_parameter("seq", [S, D], f32, isOutput=False)
    # interleaved (s8, e8m1) per window, replicated across partitions
    seb = nc.declare_dram_parameter("seb", [P, NW * 2 * P], i8, isOutput=False)
    wrec = nc.declare_dram_parameter("wrec", [P, JG], f32, isOutput=False)
    out = nc.declare_dram_parameter("out", [N_SPANS, D], f32, isOutput=True)

    wbase = []
    w0 = 0
    for j in range(JG):
        wbase.append(w0)
        w0 += windows[j][1]

    with TileContext(nc) as tc:
        with (
            tc.tile_pool(name="const", bufs=1) as const_pool,
            tc.tile_pool(name="x", bufs=4) as x_pool,
            tc.tile_pool(name="a", bufs=4) as a_pool,
            tc.tile_pool(name="ps", bufs=4, space="PSUM") as ps_pool,
            tc.tile_pool(name="misc", bufs=1) as misc_pool,
            tc.tile_pool(name="res", bufs=3) as res_pool,
        ):
            iota = const_pool.tile([P, 1], i32)
            nc.gpsimd.iota(iota[:], pattern=[[1, 1]], base=0, channel_multiplier=1)
            tcol = const_pool.tile([P, 1], f32)
            nc.gpsimd.tensor_copy(out=tcol[:], in_=iota[:])

            WR = misc_pool.tile([P, JG], f32)
            nc.scalar.dma_start(out=WR[:], in_=wrec[:])
            # bounds ride the ACT ring, one DMA per chunk so chunk 0's
            # compares start early
            SEB = misc_pool.tile([P, NW * 2 * P], i8)
            for j in range(JG):
                lo = wbase[j] * 2 * P
                hi = (wbase[j] + windows[j][1]) * 2 * P
                nc.scalar.dma_start(out=SEB[:, lo:hi], in_=seb[:, lo:hi])

            GB = 4
            NG = NBLK // GB
            XH = misc_pool.tile([P, NBLK * D], f16)
            MTbig = misc_pool.tile([P, NW, P], f16)

            bigxs = []
            for g in range(NG):
                t0 = g * GB * P
                bigx = x_pool.tile([P, GB * D], f32)
                nc.sync.dma_start(
                    out=bigx[:],
                    in_=seq[t0 : t0 + GB * P, :].rearrange(
                        "(m p) d -> p m d", p=P
                    ),
                )
                bigxs.append(bigx)

            def emit_cast(g):
                nc.scalar.activation(
                    out=XH[:, g * GB * D : (g + 1) * GB * D],
                    in_=bigxs[g][:], func=Act.Copy,
                )

            def emit_indicators(j):
                b0, kj = windows[j]
                for bb in range(kj):
                    ww = wbase[j] + bb
                    s8 = SEB[:, ww * 2 * P : (ww + 1) * 2 * P : 2]
                    e8 = SEB[:, ww * 2 * P + 1 : (ww + 1) * 2 * P : 2]
                    A = a_pool.tile([P, P], f16, name=f"A{ww}")
                    nc.vector.tensor_scalar(
                        out=A[:], in0=s8, scalar1=tcol[:, 0:1],
                        scalar2=None, op0=Alu.is_le,
                    )
                    nc.vector.scalar_tensor_tensor(
                        out=MTbig[:, ww, :], in0=e8, scalar=tcol[:, 0:1],
                        in1=A[:], op0=Alu.is_ge, op1=Alu.mult,
                    )

            def emit_mms(j):
                b0, kj = windows[j]
                ps = ps_pool.tile([P, D], f32)
                for bb in range(kj):
                    blk = b0 + bb
                    nc.tensor.matmul(
                        out=ps[:],
                        lhsT=MTbig[:, wbase[j] + bb, :],
                        rhs=XH[:, blk * D : (blk + 1) * D],
                        start=(bb == 0), stop=(bb == kj - 1),
                    )
                rj = res_pool.tile([P, D], f32)
                nc.scalar.activation(
                    out=rj[:], in_=ps[:], func=Act.Copy,
                    scale=WR[:, j : j + 1],
                )
                oj = out[:].rearrange("(c p) d -> p c d", p=P)[:, j, :]
                nc.scalar.dma_start(out=oj, in_=rj[:])

            emit_cast(0)
            emit_cast(1)
            emit_indicators(0)
            for j in range(JG):
                if j + 2 < NG:
                    emit_cast(j + 2)
                if j + 1 < JG:
                    emit_indicators(j + 1)
                emit_mms(j)
    nc.finalize()
    return nc


def _make_in_maps(sequence_tensor, si, perms, windows, ss, ee):
    seq = np.ascontiguousarray(np.asarray(sequence_tensor), dtype=np.float32)
    NW = sum(k for _, k in windows)
    in_maps = []
    for b in range(B):
        seb_rows = np.empty(NW * 2 * P, dtype=np.int8)
        w = 0
        for j in range(JG):
            b0, kj = windows[j]
            cs = ss[b, j * P : (j + 1) * P]
            ce1 = ee[b, j * P : (j + 1) * P] - 1
            for bb in range(kj):
                base = 128 * (b0 + bb)
                sh = cs - base
                eh = ce1 - base
                miss = (eh < 0) | (sh > 127)
                s8 = np.clip(sh, -128, 127)
                e8 = np.clip(eh, -128, 127)
                s8[miss] = 127
                e8[miss] = -128
                seb_rows[w * 2 * P : (w + 1) * 2 * P : 2] = s8
                seb_rows[w * 2 * P + 1 : (w + 1) * 2 * P : 2] = e8
                w += 1
        seb = np.tile(seb_rows, (P, 1))
        wr = (
            1.0 / (ee[b] - ss[b]).astype(np.float32)
        ).reshape(JG, P).T.copy()
        in_maps.append({"seq": seq[b], "seb": seb, "wrec": wr})
    return in_maps


def kernel(sequence_tensor, span_indices):
    from concourse.bass_utils import run_bass_kernel_spmd

    si = np.asarray(span_indices)
    assert si.shape == (B, N_SPANS, 2)
    key = si.tobytes()
    if _cache["key"] != key:
        perms, windows, ss, ee = _plan_windows(si)
        _cache.update(
            key=key, nc=build_nc(windows),
            plan=(perms, windows, ss, ee),
        )
    perms, windows, ss, ee = _cache["plan"]
    in_maps = _make_in_maps(sequence_tensor, si, perms, windows, ss, ee)
    res = run_bass_kernel_spmd(_cache["nc"], in_maps, list(range(B)))
    full = np.empty((B, N_SPANS, D), dtype=np.float32)
    for b in range(B):
        full[b, perms[b], :] = res.results[b]["out"]
    return full


# revision 25
# speedup vs baseline: 1.4291x; 1.0822x over previous
"""Trainium2 Bass kernel for AverageSpanExtractor (segment mean over spans).

Math note: the reference's masked softmax over all-ones logits reduces
exactly to a mean over the span tokens [start, end):
    out[b, n, :] = mean(sequence_tensor[b, start:end, :]).

Strategy (8 cores, batch-parallel — one batch element per core), built
around sorted-span segment matmuls. Indexed-fetch approaches (SWDGE
gather, ap_gather, indirect_copy) all bottom out at >=20us for the 3k
random row fetches this problem needs; the PE is power-throttled to
~1.2 GHz with ~180ns fixed cost per matmul, so the design minimizes PE
instruction count:

  1. HOST: sort each batch's spans by start. A 128-span chunk of the
     sorted order covers a ~640-token window (5-7 of the 32 128-token
     blocks). Window bounds are unioned across the 8 cores so one SPMD
     program serves all; the nc is built (and cached) per
     span-structure — exact for the given inputs, correct for any.
  2. HOST ships per-chunk span bounds (s, e-1 shifted by the chunk's
     first block) as int16, replicated across partitions: 0.5 MB.
  3. DEVICE: per window (chunk j, block b), the token-major indicator
        MT[t, i] = (s16[i] <= tg) * (e16m1[i] >= tg),  tg = t + 128*b'
     builds with two fused DVE compares against a per-window column of
     the block-shifted iota table — no PE transposes, no gathers.
     Per chunk, K_j matmuls accumulate
        out_j += MT.T @ x_block        (f16, f32 PSUM)
     chasing the f32->f16 casts of the streamed sequence.
  4. Scale rows by 1/w during the PSUM->SBUF copy (scalar engine,
     activation scale), store contiguous; HOST unpermutes rows.

Precision: binary f16 indicator is exact; x quantized to f16 (2^-11)
=> ~2e-4 global rel err.
"""

import numpy as np

B, S, D = 8, 4096, 256
N_SPANS = 1024
P = 128
NBLK = S // P
JG = N_SPANS // P      # 8 span chunks of 128

_cache = {"key": None}


def _plan_windows(si):
    """Sorted-span chunk block windows, unioned across cores."""
    perms = np.empty((B, N_SPANS), dtype=np.int64)
    ss = np.empty((B, N_SPANS), dtype=np.int64)
    ee = np.empty((B, N_SPANS), dtype=np.int64)
    for b in range(B):
        perm = np.argsort(si[b, :, 0], kind="stable")
        perms[b] = perm
        ss[b] = si[b, perm, 0]
        ee[b] = si[b, perm, 1]
    windows = []
    for j in range(JG):
        b0 = NBLK
        b1 = 0
        for b in range(B):
            cs = ss[b, j * P : (j + 1) * P]
            ce = ee[b, j * P : (j + 1) * P]
            b0 = min(b0, int(cs.min()) >> 7)
            b1 = max(b1, (int(ce.max()) - 1) >> 7)
        windows.append((b0, b1 - b0 + 1))
    return perms, windows, ss, ee


def build_nc(windows):
    import concourse.bacc as bacc
    import concourse.mybir as mybir
    from concourse.tile import TileContext

    f32 = mybir.dt.float32
    f16 = mybir.dt.float16
    i16 = mybir.dt.int16
    i32 = mybir.dt.int32
    Alu = mybir.AluOpType
    Act = mybir.ActivationFunctionType

    KMAX = max(k for _, k in windows)

    nc = bacc.Bacc(None, target_bir_lowering=False, debug=False, num_devices=B)
    seq = nc.declare_dram_parameter("seq", [S, D], f32, isOutput=False)
    # per-chunk block-shifted bounds (s, e-1), replicated across partitions
    scd = nc.declare_dram_parameter("scd", [P, N_SPANS], i16, isOutput=False)
    ecd = nc.declare_dram_parameter("ecd", [P, N_SPANS], i16, isOutput=False)
    wrec = nc.declare_dram_parameter("wrec", [P, JG], f32, isOutput=False)
    out = nc.declare_dram_parameter("out", [N_SPANS, D], f32, isOutput=True)

    wbase = []
    w0 = 0
    for j in range(JG):
        wbase.append(w0)
        w0 += windows[j][1]
    NW = w0

    with TileContext(nc) as tc:
        with (
            tc.tile_pool(name="const", bufs=1) as const_pool,
            tc.tile_pool(name="x", bufs=4) as x_pool,
            tc.tile_pool(name="a", bufs=4) as a_pool,
            tc.tile_pool(name="ps", bufs=4, space="PSUM") as ps_pool,
            tc.tile_pool(name="misc", bufs=1) as misc_pool,
            tc.tile_pool(name="res", bufs=3) as res_pool,
        ):
            # TB[p, c] = p + 128*c  (token id of row p in window-block c)
            tbi = const_pool.tile([P, KMAX], i32)
            nc.gpsimd.iota(
                tbi[:], pattern=[[P, KMAX]], base=0, channel_multiplier=1
            )
            TB = const_pool.tile([P, KMAX], f32)
            nc.gpsimd.tensor_copy(out=TB[:], in_=tbi[:])

            SC = misc_pool.tile([P, N_SPANS], i16)
            nc.sync.dma_start(out=SC[:], in_=scd[:])
            EC = misc_pool.tile([P, N_SPANS], i16)
            nc.sync.dma_start(out=EC[:], in_=ecd[:])
            WR = misc_pool.tile([P, JG], f32)
            nc.scalar.dma_start(out=WR[:], in_=wrec[:])

            GB = 4
            NG = NBLK // GB
            XH = misc_pool.tile([P, NBLK * D], f16)
            MTbig = misc_pool.tile([P, NW, P], f16)

            bigxs = []
            for g in range(NG):
                t0 = g * GB * P
                bigx = x_pool.tile([P, GB * D], f32)
                nc.sync.dma_start(
                    out=bigx[:],
                    in_=seq[t0 : t0 + GB * P, :].rearrange(
                        "(m p) d -> p m d", p=P
                    ),
                )
                bigxs.append(bigx)

            def emit_cast(g):
                nc.scalar.activation(
                    out=XH[:, g * GB * D : (g + 1) * GB * D],
                    in_=bigxs[g][:], func=Act.Copy,
                )

            def emit_indicators(j):
                b0, kj = windows[j]
                sj = SC[:, j * P : (j + 1) * P]
                ej = EC[:, j * P : (j + 1) * P]
                for bb in range(kj):
                    ww = wbase[j] + bb
                    A = a_pool.tile([P, P], f16, name=f"A{ww}")
                    nc.vector.tensor_scalar(
                        out=A[:], in0=sj, scalar1=TB[:, bb : bb + 1],
                        scalar2=None, op0=Alu.is_le,
                    )
                    nc.vector.scalar_tensor_tensor(
                        out=MTbig[:, ww, :], in0=ej, scalar=TB[:, bb : bb + 1],
                        in1=A[:], op0=Alu.is_ge, op1=Alu.mult,
                    )

            def emit_mms(j):
                b0, kj = windows[j]
                ps = ps_pool.tile([P, D], f32)
                for bb in range(kj):
                    blk = b0 + bb
                    nc.tensor.matmul(
                        out=ps[:],
                        lhsT=MTbig[:, wbase[j] + bb, :],
                        rhs=XH[:, blk * D : (blk + 1) * D],
                        start=(bb == 0), stop=(bb == kj - 1),
                    )
                rj = res_pool.tile([P, D], f32)
                nc.scalar.activation(
                    out=rj[:], in_=ps[:], func=Act.Copy,
                    scale=WR[:, j : j + 1],
                )
                oj = out[:].rearrange("(c p) d -> p c d", p=P)[:, j, :]
                nc.scalar.dma_start(out=oj, in_=rj[:])

            emit_cast(0)
            emit_cast(1)
            emit_indicators(0)
            for j in range(JG):
                if j + 2 < NG:
                    emit_cast(j + 2)
                if j + 1 < JG:
                    emit_indicators(j + 1)
                emit_mms(j)
    nc.finalize()
    return nc


def _make_in_maps(sequence_tensor, si, perms, windows, ss, ee):
    seq = np.ascontiguousarray(np.asarray(sequence_tensor), dtype=np.float32)
    in_maps = []
    for b in range(B):
        sc = np.empty(N_SPANS, dtype=np.int16)
        ec = np.empty(N_SPANS, dtype=np.int16)
        for j in range(JG):
            b0 = windows[j][0]
            sl = slice(j * P, (j + 1) * P)
            sc[sl] = ss[b, sl] - 128 * b0
            ec[sl] = ee[b, sl] - 1 - 128 * b0
        wr = (
            1.0 / (ee[b] - ss[b]).astype(np.float32)
        ).reshape(JG, P).T.copy()
        in_maps.append(
            {
                "seq": seq[b],
                "scd": np.tile(sc, (P, 1)),
                "ecd": np.tile(ec, (P, 1)),
                "wrec": wr,
            }
        )
    return in_maps


def kernel(sequence_tensor, span_indices):
    from concourse.bass_utils import run_bass_kernel_spmd

    si = np.asarray(span_indices)
    assert si.shape == (B, N_SPANS, 2)
    key = si.tobytes()
    if _cache["key"] != key:
        perms, windows, ss, ee = _plan_windows(si)
        _cache.update(
            key=key, nc=build_nc(windows),
            plan=(perms, windows, ss, ee),
        )
    perms, windows, ss, ee = _cache["plan"]
    in_maps = _make_in_maps(sequence_tensor, si, perms, windows, ss, ee)
    res = run_bass_kernel_spmd(_cache["nc"], in_maps, list(range(B)))
    full = np.empty((B, N_SPANS, D), dtype=np.float32)
    for b in range(B):
        full[b, perms[b], :] = res.results[b]["out"]
    return full
